# revision 9
# baseline (speedup 1.0000x reference)
# Conv2d 3x3 SAME (stride 1) on Trainium2, data-parallel over batch on 8 cores.
#
# Full problem: x[16, 64, 256, 256] f32, weight[128, 64, 3, 3], bias[128]
#   -> out[16, 128, 256, 256] f32.
#
# Per-core kernel (2 images/core): conv lowered to shift-and-matmul.
#   - Host pre-pads x with the zero border -> xp[bpc, 64, 258, 258], so a
#     tap (kh, kw) is just an AP offset into the SBUF strip and no edge
#     handling exists on device at all.
#   - PE: every tap is a K=64 matmul (C_IN=64), issued as CONCURRENT
#     row-group pairs: group g's tap runs on partitions 0..63
#     (tile_position (0,0)) while group g+1's same tap runs on partitions
#     64..127 (tile_position (64,0)), which hold a copy of x shifted
#     forward one row. Measured on HW: such h0/h1 duals issue with ~0ns
#     gap (~110ns per K=64 N=512 matmul, full-array utilization), while
#     serial K=64 matmuls run at 427ns and K=128 at 220ns. 9 dual slots
#     per 2-row-group pair = the same throughput as an ideal K=128
#     pipeline with zero wasted array cycles and no PE mode switches.
#   - operands in fp16 (11-bit effective mantissa, same precision class as
#     the hardware's TF32-ish f32r mode which measured ~3 cyc/row): native
#     2-byte PE path at 1 cycle/row, FWL weight loads, half the x DMA
#     bytes. Accumulation is fp32 in PSUM; output fp32.
#   - PSUM accumulates the 6 taps per 2-row output group (N = 2*256 = 512,
#     one PSUM bank); DVE evicts PSUM->SBUF fused with the bias add.

import numpy as np

import concourse.bass as bass
import concourse.mybir as mybir
import concourse.tile as tile
from concourse import bacc
from concourse.bass_utils import run_bass_kernel_spmd

N_CORES = 8
B, C_IN, H, W = 16, 64, 256, 256
C_OUT = 128
BPC = B // N_CORES  # images per core

F16 = mybir.dt.float16
F32 = mybir.dt.float32


def build_nc(bpc=BPC, h=H, w=W, rstrip=16, packed=True):
    """Build the per-core Bass module. Every core runs this same program on
    its own slice of the batch. Input xp is the host-padded image
    [bpc, C_IN, h+2, w+2] (zero border)."""
    assert h % rstrip == 0 and rstrip % 2 == 0
    wp = w + 2
    nc = bacc.Bacc("TRN2", target_bir_lowering=False, debug=False)

    # h+3 padded rows: row 0 zero, rows 1..h = x, rows h+1 and h+2 zero (the
    # extra bottom row lets the shifted upper-half load stay in bounds).
    xp_d = nc.dram_tensor("xp", [bpc, C_IN, h + 3, wp], F16, kind="ExternalInput")
    if packed:
        # all 9 taps, replicated into both partition halves: [2*C_IN, 9, C_OUT]
        wall_d = nc.dram_tensor("wall", [2 * C_IN, 9, C_OUT], F16, kind="ExternalInput")
        wsing_d = None
    else:
        wsing_d = nc.dram_tensor("wsing", [C_IN, 9, C_OUT], F16, kind="ExternalInput")
        wall_d = None
    bias_d = nc.dram_tensor("bias", [C_OUT, 1], F32, kind="ExternalInput")
    y_d = nc.dram_tensor("y", [bpc, C_OUT, h, w], F32, kind="ExternalOutput")

    with tile.TileContext(nc) as tc:
        with (
            tc.tile_pool(name="consts", bufs=1) as consts,
            tc.tile_pool(name="xpool", bufs=6) as xpool,
            tc.tile_pool(name="ypool", bufs=8) as ypool,
            tc.tile_pool(name="psum", bufs=4, space="PSUM") as psum,
        ):
            if packed:
                wall_sb = consts.tile([2 * C_IN, 9, C_OUT], F16)
                nc.sync.dma_start(out=wall_sb, in_=wall_d.ap())
            else:
                wsing_sb = consts.tile([C_IN, 9, C_OUT], F16)
                nc.sync.dma_start(out=wsing_sb, in_=wsing_d.ap())
            bias_sb = consts.tile([C_OUT, 1], F32)
            nc.sync.dma_start(out=bias_sb, in_=bias_d.ap())

            for n in range(bpc):
                for r0 in range(0, h, rstrip):
                    # lower half (partitions 0..63): slot s <-> x row r0-1+s
                    # = padded row r0+s, for s in 0..rstrip+1.
                    xl = xpool.tile([128, rstrip + 2, wp], F16, tag="xl")
                    nc.sync.dma_start(
                        out=xl[0:C_IN, :, :],
                        in_=xp_d.ap()[n, :, r0 : r0 + rstrip + 2, :],
                    )
                    if packed:
                        # upper half: slot s <-> x row r0+s = padded row
                        # r0+1+s, for s in 0..rstrip+1 (one row ahead of the
                        # lower half at the same slot).
                        nc.sync.dma_start(
                            out=xl[C_IN:128, :, :],
                            in_=xp_d.ap()[n, :, r0 + 1 : r0 + rstrip + 3, :],
                        )

                    if packed:
                        for gu in range(rstrip // 4):  # 2-group units
                            ja = 4 * gu  # rows of group A (h0, lower x)
                            jb = ja + 2  # rows of group B (h1, upper x)
                            y_sb = ypool.tile([C_OUT, 4, w], F32, tag="y")
                            psa = psum.tile([C_OUT, 2, w], F32, tag="psa")
                            psb = psum.tile([C_OUT, 2, w], F32, tag="psb")
                            for t in range(9):
                                kh, kw = divmod(t, 3)
                                nc.tensor.matmul(
                                    psa,
                                    lhsT=wall_sb[0:C_IN, t, :],
                                    rhs=xl[0:C_IN, ja + kh : ja + kh + 2, kw : kw + w],
                                    start=(t == 0),
                                    stop=(t == 8),
                                    tile_position=(0, 0),
                                )
                                # upper slot s holds x row r0+s, so group B's
                                # tap kh reads slots jb+kh-1 .. jb+kh.
                                nc.tensor.matmul(
                                    psb,
                                    lhsT=wall_sb[C_IN:128, t, :],
                                    rhs=xl[C_IN:128, jb + kh - 1 : jb + kh + 1, kw : kw + w],
                                    start=(t == 0),
                                    stop=(t == 8),
                                    tile_position=(64, 0),
                                )
                            nc.vector.tensor_scalar_add(y_sb[:, 0:2, :], psa, bias_sb)
                            nc.vector.tensor_scalar_add(y_sb[:, 2:4, :], psb, bias_sb)
                            # per-unit store on the scalar HWDGE ring
                            nc.scalar.dma_start(
                                out=y_d.ap()[n, :, r0 + ja : r0 + ja + 4, :],
                                in_=y_sb,
                            )
                    else:
                        y_sb = ypool.tile([C_OUT, rstrip, w], F32, tag="y")
                        for g in range(rstrip // 2):
                            j = 2 * g
                            ps = psum.tile([C_OUT, 2, w], F32, tag="ps")
                            for t in range(9):
                                kh, kw = divmod(t, 3)
                                nc.tensor.matmul(
                                    ps,
                                    lhsT=wsing_sb[:, t, :],
                                    rhs=xl[0:C_IN, j + kh : j + kh + 2, kw : kw + w],
                                    start=(t == 0),
                                    stop=(t == 8),
                                )
                            nc.vector.tensor_scalar_add(y_sb[:, j : j + 2, :], ps, bias_sb)
                        nc.scalar.dma_start(
                            out=y_d.ap()[n, :, r0 : r0 + rstrip, :], in_=y_sb
                        )

    nc.compile()
    return nc


def pad_x(x):
    """[n, c, h, w] -> zero-bordered fp16 [n, c, h+3, w+2] (2 bottom pad
    rows; see build_nc)."""
    n, c, h, w = x.shape
    xp = np.zeros((n, c, h + 3, w + 2), np.float16)
    xp[:, :, 1 : h + 1, 1 : w + 1] = x
    return xp


def prep_weights(weight, packed=True):
    """weight [C_OUT, C_IN, 3, 3] -> lhsT layouts [ci, tap, co]."""
    wt = np.ascontiguousarray(np.transpose(weight, (1, 2, 3, 0)).astype(np.float16))  # [ci, kh, kw, co]
    w9 = wt.reshape(C_IN, 9, C_OUT)
    if packed:
        wall = np.ascontiguousarray(np.concatenate([w9, w9], axis=0))  # [2*ci, 9, co]
        return wall, None
    return None, np.ascontiguousarray(w9)


_NC_CACHE = {}
LAST_RESULT = None  # BassKernelResults of the most recent run (for test harness)
TRACE = False
PACKED = True


def kernel(x, weight, bias):
    global LAST_RESULT
    x = np.asarray(x, dtype=np.float32)
    weight = np.asarray(weight, dtype=np.float32)
    bias = np.asarray(bias, dtype=np.float32)

    key = ("full", PACKED)
    if key not in _NC_CACHE:
        _NC_CACHE[key] = build_nc(packed=PACKED)
    nc = _NC_CACHE[key]

    xp = pad_x(x)
    wall, wsing = prep_weights(weight, packed=PACKED)
    bias2 = np.ascontiguousarray(bias.reshape(C_OUT, 1))

    in_maps = []
    for c in range(N_CORES):
        m = {
            "xp": xp[c * BPC : (c + 1) * BPC],
            "bias": bias2,
        }
        if PACKED:
            m["wall"] = wall
        else:
            m["wsing"] = wsing
        in_maps.append(m)

    res = run_bass_kernel_spmd(nc, in_maps, core_ids=list(range(N_CORES)), trace=TRACE)
    LAST_RESULT = res
    out = np.concatenate([r["y"] for r in res.results], axis=0)
    return out
